# revision 4
# baseline (speedup 1.0000x reference)
"""Cross-attention layer kernel for 8 Trainium2 NeuronCores.

Reference computation (fp32, D=1024, S=2048, B=4):
    q = x @ Wq.T + bq ; k = x @ Wk.T + bk ; v = x @ Wv.T + bv
    attn = softmax(q @ k.T / 32)
    vision = attn @ v                      # [B,S,D]
    text   = attn.T @ x                    # [B,S,D]

Sharding: core c handles batch b=c//2, query-half h=c%2 (1024 queries),
duplicating the K/V projections within each core pair.  Key order inside a
core is [own-half rows, other-half rows] so the program is static; the host
unpermutes when gathering.  The text output is computed transposed
(textT = x_scaled.T @ P) and each pair's partials are summed on the host.

All big matmuls run as float32r (fp32 storage, 8e11m read by the PE —
full rate at N=512 vs 1/4 rate for fp32).  Tensors feeding f32r matmuls
are declared float32r so every producer rounds on write; host inputs are
pre-rounded to the same grid.  Softmax skips max-subtraction (scores here
are bounded by ~3: x ~ N(0,1), W ~ U(-1/32,1/32) keep q.k/32 tiny), and
the 1/rowsum normalization is folded into the two outputs.
"""

import sys

import numpy as np

try:
    import concourse.bass as bass
except ImportError:  # pragma: no cover - grading env should have it on path
    sys.path.insert(0, "/opt/trn_rl_repo")
    import concourse.bass as bass

import concourse.mybir as mybir
import concourse.tile as tile
from concourse import bacc
from concourse.bass_utils import run_bass_kernel_spmd
from concourse.masks import make_identity

F32 = mybir.dt.float32
F32R = mybir.dt.float32r

B = 4          # batches
S = 2048       # sequence length
D = 1024       # model dim
SH = S // 2    # queries per core
P = 128        # partitions
NT = D // P    # 8 tiles along d/e
NQ = SH // P   # 8 q-tiles per core
NK = S // P    # 16 k-tiles
SCALE = 1.0 / 32.0  # 1/sqrt(D)
N512 = 512


def round_f32r(a):
    """Round-to-nearest fp32 -> fp32r (top 20 bits: 1s + 8e + 11m)."""
    u = np.ascontiguousarray(a, dtype=np.float32).view(np.uint32).astype(np.uint64)
    u = (u + 0x7FF + ((u >> 12) & 1)) & 0xFFFFF000
    return u.astype(np.uint32).view(np.float32)


def build_program():
    nc = bacc.Bacc("TRN2", target_bir_lowering=False, debug=False, num_devices=8)

    xq_h = nc.dram_tensor("xq", [SH, D], F32R, kind="ExternalInput")
    xo_h = nc.dram_tensor("xo", [SH, D], F32R, kind="ExternalInput")
    wqt_h = nc.dram_tensor("wqt", [D, D], F32R, kind="ExternalInput")
    wkt_h = nc.dram_tensor("wkt", [D, D], F32R, kind="ExternalInput")
    wvt_h = nc.dram_tensor("wvt", [D, D], F32R, kind="ExternalInput")
    bq_h = nc.dram_tensor("bq", [D], F32, kind="ExternalInput")
    bk_h = nc.dram_tensor("bk", [D], F32, kind="ExternalInput")
    bv_h = nc.dram_tensor("bv", [D], F32, kind="ExternalInput")

    vision_h = nc.dram_tensor("vision", [SH, D], F32, kind="ExternalOutput")
    textT_h = nc.dram_tensor("textT", [D, S], F32, kind="ExternalOutput")

    # tiled DRAM views
    xq_r = xq_h.ap().rearrange("(i p) d -> i p d", p=P)      # [8,128,1024]
    xo_r = xo_h.ap().rearrange("(i p) d -> i p d", p=P)
    wq_r = wqt_h.ap().rearrange("(t p) e -> p t e", p=P)     # [128,8,1024]
    wk_r = wkt_h.ap().rearrange("(t p) e -> p t e", p=P)
    wv_r = wvt_h.ap().rearrange("(t p) e -> p t e", p=P)
    bq_r = bq_h.ap().rearrange("(t p) -> p t", p=P)          # [128,8]
    bk_r = bk_h.ap().rearrange("(t p) -> p t", p=P)

    bv_ap = bv_h.ap()
    bv_bcast_src = bass.AP(tensor=bv_ap.tensor, offset=bv_ap.offset,
                           ap=[[0, P], bv_ap.ap[0]])         # [128,1024] bcast

    with tile.TileContext(nc) as tc:
        with (
            tc.tile_pool(name="big", bufs=1) as big_pool,
            tc.tile_pool(name="singles", bufs=1) as singles,
            tc.tile_pool(name="dram", bufs=1, space="DRAM") as dram_pool,
        ):
            qt_d = dram_pool.tile([D, SH], F32R)   # Q^T spill [e, q]
            v_d = dram_pool.tile([S, D], F32R)     # V spill [k, e]
            qt_r = qt_d.rearrange("(t p) q -> p t q", p=P)   # [128,8,1024]
            vh_r = v_d.rearrange("(i p) e -> p i e", p=P)    # [128,16,1024]

            ident_f = singles.tile([P, P], F32)
            make_identity(nc, ident_f)
            ident = singles.tile([P, P], F32R)
            nc.vector.tensor_copy(ident, ident_f)
            bq_sb = singles.tile([P, NT], F32)
            nc.sync.dma_start(out=bq_sb, in_=bq_r)
            bk_sb = singles.tile([P, NT], F32)
            nc.sync.dma_start(out=bk_sb, in_=bk_r)
            bvb = singles.tile([P, D], F32)
            nc.sync.dma_start(out=bvb, in_=bv_bcast_src)
            r_all = singles.tile([P, NQ], F32)

            # ---- phase 1: x^T via PE transposes -------------------------
            # xT[p, t, s] = x[s, 128t+p]; column order [own half | other]
            xT = big_pool.tile([P, NT, S], F32R, tag="bigA")
            with (
                tc.tile_pool(name="ph1_in", bufs=3) as ph1_in,
                tc.tile_pool(name="ph1_ps", bufs=4, space="PSUM") as ph1_ps,
            ):
                for src_r, off in ((xq_r, 0), (xo_r, SH)):
                    for i in range(NQ):
                        xin = ph1_in.tile([P, D], F32R, tag="xin")
                        nc.sync.dma_start(out=xin, in_=src_r[i])
                        for t in range(NT):
                            ps = ph1_ps.tile([P, P], F32R, tag="tr")
                            nc.tensor.transpose(ps, xin[:, t * P:(t + 1) * P], ident)
                            col = off + i * P
                            nc.any.tensor_copy(out=xT[:, t, col:col + P], in_=ps)

            # ---- phase 2: projections -----------------------------------
            # Q^T -> DRAM spill, K^T -> SBUF resident, V -> DRAM spill
            kT = big_pool.tile([P, NT, S], F32R, tag="bigB")
            with (
                tc.tile_pool(name="wpool", bufs=2) as wpool,
                tc.tile_pool(name="ph2_ps", bufs=4, space="PSUM") as ph2_ps,
                tc.tile_pool(name="ph2_ev", bufs=4) as ph2_ev,
            ):
                # Q^T: lhsT = Wq^T tile [128d,128e], rhs = xT own columns
                for h in range(2):
                    wt = wpool.tile([P, NT, N512], F32R, tag="wh")
                    nc.sync.dma_start(out=wt, in_=wq_r[:, :, h * N512:(h + 1) * N512])
                    for tl in range(4):
                        t = h * 4 + tl
                        for n in range(2):
                            ps = ph2_ps.tile([P, N512], F32, tag="acc")
                            for td in range(NT):
                                nc.tensor.matmul(
                                    ps,
                                    wt[:, td, tl * P:(tl + 1) * P],
                                    xT[:, td, n * N512:(n + 1) * N512],
                                    start=(td == 0), stop=(td == NT - 1))
                            ev = ph2_ev.tile([P, N512], F32R, tag="ev")
                            nc.scalar.activation(
                                ev, ps, mybir.ActivationFunctionType.Identity,
                                bias=bq_sb[:, t:t + 1], scale=1.0)
                            nc.sync.dma_start(
                                out=qt_d[t * P:(t + 1) * P,
                                         n * N512:(n + 1) * N512],
                                in_=ev)
                # K^T (full sequence) straight into SBUF
                for h in range(2):
                    wt = wpool.tile([P, NT, N512], F32R, tag="wh")
                    nc.sync.dma_start(out=wt, in_=wk_r[:, :, h * N512:(h + 1) * N512])
                    for tl in range(4):
                        t = h * 4 + tl
                        for kc in range(S // N512):
                            ps = ph2_ps.tile([P, N512], F32, tag="acc")
                            for td in range(NT):
                                nc.tensor.matmul(
                                    ps,
                                    wt[:, td, tl * P:(tl + 1) * P],
                                    xT[:, td, kc * N512:(kc + 1) * N512],
                                    start=(td == 0), stop=(td == NT - 1))
                            nc.scalar.activation(
                                kT[:, t, kc * N512:(kc + 1) * N512], ps,
                                mybir.ActivationFunctionType.Identity,
                                bias=bk_sb[:, t:t + 1], scale=1.0)
                # V natural [k, e]: lhsT = xT tile, rhs = Wv^T -> spill
                for h in range(2):
                    wt = wpool.tile([P, NT, N512], F32R, tag="wh")
                    nc.sync.dma_start(out=wt, in_=wv_r[:, :, h * N512:(h + 1) * N512])
                    for i in range(NK):
                        ps = ph2_ps.tile([P, N512], F32, tag="acc")
                        for td in range(NT):
                            nc.tensor.matmul(
                                ps,
                                xT[:, td, i * P:(i + 1) * P],
                                wt[:, td, :],
                                start=(td == 0), stop=(td == NT - 1))
                        ev = ph2_ev.tile([P, N512], F32R, tag="ev")
                        nc.vector.tensor_add(
                            ev, ps, bvb[:, h * N512:(h + 1) * N512])
                        nc.sync.dma_start(
                            out=v_d[i * P:(i + 1) * P,
                                    h * N512:(h + 1) * N512],
                            in_=ev)

            # ---- phase 3: scores + softmax (P = exp(s/32), unnormalized)
            # P_sb[p, j, k] = exp(scale * (q_{128j+p} . k_k))
            P_sb = big_pool.tile([P, NQ, S], F32R, tag="bigA")
            with (
                tc.tile_pool(name="ph3_qt", bufs=2) as ph3_qt,
                tc.tile_pool(name="ph3_ps", bufs=4, space="PSUM") as ph3_ps,
                tc.tile_pool(name="ph3_l", bufs=4) as ph3_l,
            ):
                for j in range(NQ):
                    qt = ph3_qt.tile([P, NT, P], F32R, tag="qt")
                    nc.sync.dma_start(out=qt, in_=qt_r[:, :, j * P:(j + 1) * P])
                    l4 = ph3_l.tile([P, S // N512], F32, tag="l4")
                    for kc in range(S // N512):
                        ps = ph3_ps.tile([P, N512], F32, tag="s")
                        for t in range(NT):
                            nc.tensor.matmul(
                                ps,
                                qt[:, t, :],
                                kT[:, t, kc * N512:(kc + 1) * N512],
                                start=(t == 0), stop=(t == NT - 1))
                        nc.scalar.activation(
                            P_sb[:, j, kc * N512:(kc + 1) * N512], ps,
                            mybir.ActivationFunctionType.Exp,
                            bias=0.0, scale=SCALE,
                            accum_out=l4[:, kc:kc + 1])
                    lsum = ph3_l.tile([P, 1], F32, tag="lsum")
                    nc.vector.reduce_sum(out=lsum, in_=l4,
                                         axis=mybir.AxisListType.X)
                    nc.vector.reciprocal(out=r_all[:, j:j + 1], in_=lsum)

            # ---- phase 4: P^T via PE transposes -------------------------
            PT = big_pool.tile([P, NK, SH], F32R, tag="bigB")
            with tc.tile_pool(name="ph4_ps", bufs=4, space="PSUM") as ph4_ps:
                for j in range(NQ):
                    for i in range(NK):
                        ps = ph4_ps.tile([P, P], F32R, tag="tr")
                        nc.tensor.transpose(
                            ps, P_sb[:, j, i * P:(i + 1) * P], ident)
                        nc.any.tensor_copy(
                            out=PT[:, i, j * P:(j + 1) * P], in_=ps)

            # ---- phase 5: vision = rowscale(P @ V) ----------------------
            with (
                tc.tile_pool(name="ph5_v", bufs=2) as ph5_v,
                tc.tile_pool(name="ph5_ps", bufs=8, space="PSUM") as ph5_ps,
                tc.tile_pool(name="ph5_ev", bufs=4) as ph5_ev,
            ):
                for h in range(2):
                    vh = ph5_v.tile([P, NK, N512], F32R, tag="vh")
                    nc.sync.dma_start(
                        out=vh, in_=vh_r[:, :, h * N512:(h + 1) * N512])
                    for j in range(NQ):
                        ps = ph5_ps.tile([P, N512], F32, tag="vp")
                        for i in range(NK):
                            nc.tensor.matmul(
                                ps,
                                PT[:, i, j * P:(j + 1) * P],
                                vh[:, i, :],
                                start=(i == 0), stop=(i == NK - 1))
                        ev = ph5_ev.tile([P, N512], F32, tag="ev")
                        nc.vector.tensor_scalar_mul(ev, ps, r_all[:, j:j + 1])
                        nc.sync.dma_start(
                            out=vision_h.ap()[j * P:(j + 1) * P,
                                              h * N512:(h + 1) * N512],
                            in_=ev)

            # ---- phase 6: textT = (x_q * r).T @ P -----------------------
            with (
                tc.tile_pool(name="ph6_xs", bufs=1) as ph6_xs,
                tc.tile_pool(name="ph6_in", bufs=2) as ph6_in,
                tc.tile_pool(name="ph6_ps", bufs=8, space="PSUM") as ph6_ps,
                tc.tile_pool(name="ph6_ev", bufs=4) as ph6_ev,
            ):
                xs = ph6_xs.tile([P, NQ, D], F32R, tag="xs")
                for j in range(NQ):
                    xin = ph6_in.tile([P, D], F32R, tag="xin")
                    nc.sync.dma_start(out=xin, in_=xq_r[j])
                    nc.vector.tensor_scalar_mul(
                        xs[:, j, :], xin, r_all[:, j:j + 1])
                for kc in range(S // N512):
                    for dc in range(NT):
                        ps = ph6_ps.tile([P, N512], F32, tag="tp")
                        for j in range(NQ):
                            nc.tensor.matmul(
                                ps,
                                xs[:, j, dc * P:(dc + 1) * P],
                                P_sb[:, j, kc * N512:(kc + 1) * N512],
                                start=(j == 0), stop=(j == NQ - 1))
                        ev = ph6_ev.tile([P, N512], F32, tag="ev")
                        nc.any.tensor_copy(out=ev, in_=ps)
                        nc.sync.dma_start(
                            out=textT_h.ap()[dc * P:(dc + 1) * P,
                                             kc * N512:(kc + 1) * N512],
                            in_=ev)

    nc.compile()
    return nc


_NC_CACHE = []


def _get_program():
    if not _NC_CACHE:
        _NC_CACHE.append(build_program())
    return _NC_CACHE[0]


def kernel(inputs, Wq, bq, Wk, bk, Wv, bv, _run_opts=None):
    x = round_f32r(np.asarray(inputs, dtype=np.float32))
    WqT = round_f32r(np.asarray(Wq, dtype=np.float32).T)
    WkT = round_f32r(np.asarray(Wk, dtype=np.float32).T)
    WvT = round_f32r(np.asarray(Wv, dtype=np.float32).T)
    bq = np.ascontiguousarray(np.asarray(bq, dtype=np.float32))
    bk = np.ascontiguousarray(np.asarray(bk, dtype=np.float32))
    bv = np.ascontiguousarray(np.asarray(bv, dtype=np.float32))

    nc = _get_program()

    in_maps = []
    for c in range(8):
        b, h = divmod(c, 2)
        xq = np.ascontiguousarray(x[b, h * SH:(h + 1) * SH])
        xo = np.ascontiguousarray(x[b, (1 - h) * SH:(2 - h) * SH])
        in_maps.append({
            "xq": xq, "xo": xo,
            "wqt": WqT, "wkt": WkT, "wvt": WvT,
            "bq": bq, "bk": bk, "bv": bv,
        })

    run_opts = dict(_run_opts or {})
    res = run_bass_kernel_spmd(nc, in_maps, core_ids=list(range(8)), **run_opts)
    results = res.results

    vision = np.empty((B, S, D), np.float32)
    text = np.zeros((B, S, D), np.float32)
    for c in range(8):
        b, h = divmod(c, 2)
        vision[b, h * SH:(h + 1) * SH] = results[c]["vision"]
        tT = results[c]["textT"]  # [D, S] with k order [own half, other half]
        text[b, h * SH:(h + 1) * SH] += tT[:, :SH].T
        text[b, (1 - h) * SH:(2 - h) * SH] += tT[:, SH:].T
    if _run_opts is not None:
        return (vision, text), res
    return (vision, text)
